# revision 1
# baseline (speedup 1.0000x reference)
"""Trainium2 Bass kernel for one-hop GNN mean aggregation + per-clip projection.

Computation (see reference):
    nodes [2048, 10] int64  -> flat n = 20480 node ids in [0, 50000)
    adj   [50000, 32] int64 -> neighbor lists
    features [50000, 256] f32
    local_weight [8, 128, 256] f32
    out[n, c, k] = relu( mean_j features[adj[nodes[n], j]] @ local_weight[c, k, :] )
    returned as [2048, 10, 8, 128] f32

Strategy: data-parallel over the 20480 flat nodes across 8 NeuronCores
(2560 nodes/core, 20 chunks of 128).  Per chunk:
  1. indirect-DMA gather of 128 adj rows ([128, 32] int32)
  2. one big indirect-DMA gather of 128*32 feature rows -> [128, 32, 256] f32
  3. DVE reduction over the 32 neighbors -> [128, 256]
  4. PE transpose (2x [128,128]) -> featT, then accumulating matmul against
     host-pretransposed W^T [256, 1024]
  5. fused (1/32 scale + ReLU) on ACT straight out of PSUM
  6. DMA the [128, 1024] result rows back to HBM
"""

import numpy as np

import concourse.bass as bass
import concourse.bacc as bacc
import concourse.mybir as mybir
import concourse.tile as tile
from concourse.bass import IndirectOffsetOnAxis
from concourse.bass_utils import run_bass_kernel_spmd
from concourse.masks import make_identity

N_CORES = 8
NUM_NODES = 50000
FEAT_DIM = 256
CLIPS = 8
DIM = 128
K_NEIGH = 32
B, S = 2048, 10
N_FLAT = B * S                      # 20480
N_PER_CORE = N_FLAT // N_CORES      # 2560
P = 128
N_CHUNKS = N_PER_CORE // P          # 20
CK = CLIPS * DIM                    # 1024

_last_results = None  # BassKernelResults of the most recent run (for test.py)


def build_program():
    nc = bacc.Bacc(
        "TRN2",
        target_bir_lowering=False,
        debug=False,
        num_devices=N_CORES,
    )
    nodes_d = nc.dram_tensor("nodes_i32", [N_PER_CORE], mybir.dt.int32, kind="ExternalInput")
    adj_d = nc.dram_tensor("adj_i32", [NUM_NODES, K_NEIGH], mybir.dt.int32, kind="ExternalInput")
    feat_d = nc.dram_tensor("features", [NUM_NODES, FEAT_DIM], mybir.dt.float32, kind="ExternalInput")
    w_d = nc.dram_tensor("w_t", [2, P, CK], mybir.dt.float32, kind="ExternalInput")
    out_d = nc.dram_tensor("out", [N_PER_CORE, CK], mybir.dt.float32, kind="ExternalOutput")

    with tile.TileContext(nc) as tc:
        with (
            tc.tile_pool(name="const", bufs=1) as const_pool,
            tc.tile_pool(name="work", bufs=2) as work,
            tc.tile_pool(name="gath", bufs=2) as gath_pool,
            tc.tile_pool(name="psum_t", bufs=4, space=bass.MemorySpace.PSUM) as psum_t,
            tc.tile_pool(name="psum_o", bufs=2, space=bass.MemorySpace.PSUM) as psum_o,
        ):
            identity = const_pool.tile([P, P], mybir.dt.float32)
            make_identity(nc, identity[:])

            w_sb = const_pool.tile([P, 2, CK], mybir.dt.float32)
            for h in range(2):
                nc.sync.dma_start(out=w_sb[:, h, :], in_=w_d[h, :, :])

            nodes_sb = const_pool.tile([P, N_CHUNKS], mybir.dt.int32)
            nc.sync.dma_start(
                out=nodes_sb[:], in_=nodes_d.ap().rearrange("(c p) -> p c", p=P)
            )

            for ch in range(N_CHUNKS):
                adj_tile = work.tile([P, K_NEIGH], mybir.dt.int32)
                nc.gpsimd.indirect_dma_start(
                    out=adj_tile[:],
                    out_offset=None,
                    in_=adj_d[:, :],
                    in_offset=IndirectOffsetOnAxis(ap=nodes_sb[:, ch : ch + 1], axis=0),
                )

                gath = gath_pool.tile([P, K_NEIGH, FEAT_DIM], mybir.dt.float32)
                for k in range(K_NEIGH):
                    nc.gpsimd.indirect_dma_start(
                        out=gath[:, k, :],
                        out_offset=None,
                        in_=feat_d[:, :],
                        in_offset=IndirectOffsetOnAxis(ap=adj_tile[:, k : k + 1], axis=0),
                    )

                fsum = work.tile([P, FEAT_DIM], mybir.dt.float32)
                nc.vector.tensor_reduce(
                    out=fsum[:],
                    in_=gath[:].rearrange("p j d -> p d j"),
                    axis=mybir.AxisListType.X,
                    op=mybir.AluOpType.add,
                )

                featT = work.tile([P, 2, P], mybir.dt.float32)
                for h in range(2):
                    tp = psum_t.tile([P, P], mybir.dt.float32)
                    nc.tensor.transpose(
                        out=tp[:], in_=fsum[:, h * P : (h + 1) * P], identity=identity[:]
                    )
                    nc.vector.tensor_copy(out=featT[:, h, :], in_=tp[:])

                po = psum_o.tile([P, CK], mybir.dt.float32)
                for nb in range(2):
                    cols = slice(nb * 512, (nb + 1) * 512)
                    for h in range(2):
                        nc.tensor.matmul(
                            po[:, cols],
                            featT[:, h, :],
                            w_sb[:, h, cols],
                            start=(h == 0),
                            stop=(h == 1),
                        )

                out_t = work.tile([P, CK], mybir.dt.float32)
                nc.scalar.activation(
                    out=out_t[:],
                    in_=po[:],
                    func=mybir.ActivationFunctionType.Relu,
                    scale=1.0 / K_NEIGH,
                )
                nc.sync.dma_start(
                    out=out_d[ch * P : (ch + 1) * P, :], in_=out_t[:]
                )

    nc.compile()
    return nc


def prep_in_maps(nodes, adj, features, local_weight):
    nodes_flat = np.asarray(nodes).reshape(-1).astype(np.int32)
    adj_i32 = np.ascontiguousarray(np.asarray(adj).astype(np.int32))
    feat = np.ascontiguousarray(np.asarray(features).astype(np.float32))
    w = np.asarray(local_weight).astype(np.float32)
    # w_t[d, c*DIM + k] = W[c, k, d], split into the two 128-row halves
    w_t = np.ascontiguousarray(
        w.transpose(2, 0, 1).reshape(2, P, CK)
    )
    in_maps = []
    for c in range(N_CORES):
        in_maps.append(
            {
                "nodes_i32": np.ascontiguousarray(
                    nodes_flat[c * N_PER_CORE : (c + 1) * N_PER_CORE]
                ),
                "adj_i32": adj_i32,
                "features": feat,
                "w_t": w_t,
            }
        )
    return in_maps


_program_cache = None


def kernel(nodes, adj, features, local_weight, trace=False):
    global _last_results, _program_cache
    if _program_cache is None:
        _program_cache = build_program()
    nc = _program_cache
    in_maps = prep_in_maps(nodes, adj, features, local_weight)
    res = run_bass_kernel_spmd(
        nc, in_maps, core_ids=list(range(N_CORES)), trace=trace
    )
    _last_results = res
    out = np.concatenate([r["out"] for r in res.results], axis=0)
    return out.reshape(B, S, CLIPS, DIM)



# revision 8
# speedup vs baseline: 1.2249x; 1.2249x over previous
"""Trainium2 Bass kernel for one-hop GNN mean aggregation + per-clip projection.

Computation (see reference):
    nodes [2048, 10] int64  -> flat n = 20480 node ids in [0, 50000)
    adj   [50000, 32] int64 -> neighbor lists
    features [50000, 256] f32
    local_weight [8, 128, 256] f32
    out[n, c, k] = relu( mean_j features[adj[nodes[n], j]] @ local_weight[c, k, :] )
    returned as [2048, 10, 8, 128] f32

Strategy: data-parallel over the 20480 flat nodes across 8 NeuronCores
(2560 nodes/core, 20 chunks of 128).  The host resolves the two-level
index chain (adj[nodes]) and uploads per-core neighbor ids; features are
uploaded in bf16 (tolerance is 2e-2, bf16 keeps worst-case error ~0.5%)
to halve the gather traffic.

Per 128-node chunk on device:
  1. indirect-DMA gather of 128*32 bf16 feature rows (512B each)
  2. halving tree-adds on DVE (contiguous, bf16) -> neighbor sum [128, 256]
  3. PE transpose (2x [128,128] bf16) -> feat dim on partitions
  4. bf16 matmuls against host-pretransposed W^T [256, 1024] -> f32 PSUM
  5. fused (1/32 scale + ReLU) on ACT straight out of PSUM
  6. DMA the [128, 1024] f32 rows back to HBM

FLAT_GATHER selects a single indirect DMA per chunk driven by a flat
[1, 4096] offset AP living on one SBUF partition (one SWDGE instruction,
4096 descriptors); otherwise 32 per-column gathers with [128, 1] offsets.
"""

import numpy as np
import ml_dtypes

import concourse.bass as bass
import concourse.bacc as bacc
import concourse.mybir as mybir
import concourse.tile as tile
from concourse.bass import IndirectOffsetOnAxis
from concourse.bass_utils import run_bass_kernel_spmd
from concourse.masks import make_identity

N_CORES = 8
NUM_NODES = 50000
FEAT_DIM = 256
CLIPS = 8
DIM = 128
K_NEIGH = 32
B, S = 2048, 10
N_FLAT = B * S                      # 20480
N_PER_CORE = N_FLAT // N_CORES      # 2560
P = 128
N_CHUNKS = N_PER_CORE // P          # 20
CK = CLIPS * DIM                    # 1024
NIDX = P * K_NEIGH                  # 4096 rows gathered per chunk

FLAT_GATHER = False     # one [1, 4096]-offset indirect DMA per chunk (broken on HW)
FLAT_ORDER = "pmajor"   # descriptor i -> (p, k) = (i // 32, i % 32)

_last_results = None  # BassKernelResults of the most recent run (for test.py)


def build_program():
    nc = bacc.Bacc(
        "TRN2",
        target_bir_lowering=False,
        debug=False,
        num_devices=N_CORES,
    )
    feat_d = nc.dram_tensor(
        "feat_bf", [NUM_NODES, FEAT_DIM], mybir.dt.bfloat16, kind="ExternalInput"
    )
    if FLAT_GATHER:
        # chunk c's 4096 indices live on partition c
        nidx_d = nc.dram_tensor(
            "neigh_i32", [N_CHUNKS, NIDX], mybir.dt.int32, kind="ExternalInput"
        )
    else:
        # [p, c, k] = adj[nodes[c*128 + p], k]
        nidx_d = nc.dram_tensor(
            "neigh_i32", [P, N_CHUNKS, K_NEIGH], mybir.dt.int32,
            kind="ExternalInput"
        )
    w_d = nc.dram_tensor("w_t", [2, P, CK], mybir.dt.bfloat16, kind="ExternalInput")
    out_d = nc.dram_tensor(
        "out", [N_PER_CORE, CK], mybir.dt.float32, kind="ExternalOutput"
    )

    with tile.TileContext(nc) as tc:
        with (
            tc.tile_pool(name="const", bufs=1) as const_pool,
            tc.tile_pool(name="work", bufs=2) as work,
            tc.tile_pool(name="gath", bufs=3) as gath_pool,
            tc.tile_pool(name="psum_t", bufs=2, space=bass.MemorySpace.PSUM) as psum_t,
            tc.tile_pool(name="psum_o", bufs=2, space=bass.MemorySpace.PSUM) as psum_o,
        ):
            identity = const_pool.tile([P, P], mybir.dt.bfloat16)
            make_identity(nc, identity[:])

            w_sb = const_pool.tile([P, 2, CK], mybir.dt.bfloat16)
            for h in range(2):
                nc.sync.dma_start(out=w_sb[:, h, :], in_=w_d[h, :, :])

            if FLAT_GATHER:
                nidx_sb = const_pool.tile([N_CHUNKS, NIDX], mybir.dt.int32)
                nc.sync.dma_start(out=nidx_sb[:], in_=nidx_d[:, :])
            else:
                nidx_sb = const_pool.tile(
                    [P, N_CHUNKS, K_NEIGH], mybir.dt.int32
                )
                nc.sync.dma_start(
                    out=nidx_sb[:].rearrange("p c k -> p (c k)"),
                    in_=nidx_d.ap().rearrange("p c k -> p (c k)"),
                )

            for ch in range(N_CHUNKS):
                g = gath_pool.tile([P, K_NEIGH, FEAT_DIM], mybir.dt.bfloat16)
                if FLAT_GATHER:
                    nc.gpsimd.indirect_dma_start(
                        out=g[:],
                        out_offset=None,
                        in_=feat_d[:, :],
                        in_offset=IndirectOffsetOnAxis(
                            ap=nidx_sb[ch:ch + 1, :], axis=0
                        ),
                    )
                else:
                    for k in range(K_NEIGH):
                        nc.gpsimd.indirect_dma_start(
                            out=g[:, k, :],
                            out_offset=None,
                            in_=feat_d[:, :],
                            in_offset=IndirectOffsetOnAxis(
                                ap=nidx_sb[:, ch, k:k + 1], axis=0
                            ),
                        )

                # contiguous halving tree-adds: [P, 8192] -> [P, 256]
                gl = g[:].rearrange("p k d -> p (k d)")
                width = K_NEIGH * FEAT_DIM // 2
                h1 = work.tile([P, width], mybir.dt.bfloat16)
                nc.vector.tensor_tensor(
                    out=h1[:], in0=gl[:, :width], in1=gl[:, width:],
                    op=mybir.AluOpType.add,
                )
                prev = h1
                while width > 2 * FEAT_DIM:
                    width //= 2
                    nxt = work.tile([P, width], mybir.dt.bfloat16)
                    nc.vector.tensor_tensor(
                        out=nxt[:], in0=prev[:, :width], in1=prev[:, width:],
                        op=mybir.AluOpType.add,
                    )
                    prev = nxt
                fsum = work.tile([P, FEAT_DIM], mybir.dt.bfloat16)
                nc.vector.tensor_tensor(
                    out=fsum[:], in0=prev[:, :FEAT_DIM], in1=prev[:, FEAT_DIM:],
                    op=mybir.AluOpType.add,
                )

                featT = work.tile([P, 2, P], mybir.dt.bfloat16)
                for h in range(2):
                    tp = psum_t.tile([P, P], mybir.dt.bfloat16)
                    nc.tensor.transpose(
                        out=tp[:], in_=fsum[:, h * P:(h + 1) * P],
                        identity=identity[:],
                    )
                    nc.vector.tensor_copy(out=featT[:, h, :], in_=tp[:])

                po = psum_o.tile([P, CK], mybir.dt.float32)
                for nb in range(2):
                    cols = slice(nb * 512, (nb + 1) * 512)
                    for h in range(2):
                        nc.tensor.matmul(
                            po[:, cols],
                            featT[:, h, :],
                            w_sb[:, h, cols],
                            start=(h == 0),
                            stop=(h == 1),
                        )

                out_t = work.tile([P, CK], mybir.dt.float32)
                nc.scalar.activation(
                    out=out_t[:],
                    in_=po[:],
                    func=mybir.ActivationFunctionType.Relu,
                    scale=1.0 / K_NEIGH,
                )
                nc.sync.dma_start(
                    out=out_d[ch * P:(ch + 1) * P, :], in_=out_t[:]
                )

    nc.compile()
    return nc


def prep_in_maps(nodes, adj, features, local_weight):
    nodes_flat = np.asarray(nodes).reshape(-1).astype(np.int64)
    adj_np = np.asarray(adj).astype(np.int64)
    feat_bf = np.asarray(features).astype(ml_dtypes.bfloat16)
    w = np.asarray(local_weight).astype(np.float32)
    # w_t[d, c*DIM + k] = W[c, k, d], split into the two 128-row halves
    w_t = np.ascontiguousarray(
        w.transpose(2, 0, 1).reshape(2, P, CK)
    ).astype(ml_dtypes.bfloat16)

    in_maps = []
    for c in range(N_CORES):
        core_nodes = nodes_flat[c * N_PER_CORE:(c + 1) * N_PER_CORE]
        neigh = adj_np[core_nodes].astype(np.int32)      # [2560, 32]
        if FLAT_GATHER:
            byc = neigh.reshape(N_CHUNKS, P, K_NEIGH)
            if FLAT_ORDER == "pmajor":       # desc i -> (p=i//32, k=i%32)
                nidx = byc.reshape(N_CHUNKS, NIDX)
            else:                            # desc i -> (p=i%128, k=i//128)
                nidx = byc.transpose(0, 2, 1).reshape(N_CHUNKS, NIDX)
            nidx = np.ascontiguousarray(nidx)
        else:
            nidx = np.ascontiguousarray(
                neigh.reshape(N_CHUNKS, P, K_NEIGH).transpose(1, 0, 2)
            )
        in_maps.append(
            {
                "feat_bf": feat_bf,
                "neigh_i32": nidx,
                "w_t": w_t,
            }
        )
    return in_maps


_program_cache = None


def kernel(nodes, adj, features, local_weight, trace=False):
    global _last_results, _program_cache
    if _program_cache is None:
        _program_cache = build_program()
    nc = _program_cache
    in_maps = prep_in_maps(nodes, adj, features, local_weight)
    res = run_bass_kernel_spmd(
        nc, in_maps, core_ids=list(range(N_CORES)), trace=trace
    )
    _last_results = res
    out = np.concatenate([r["out"] for r in res.results], axis=0)
    return out.reshape(B, S, CLIPS, DIM)
